# revision 66
# baseline (speedup 1.0000x reference)
"""Trainium2 Bass kernel for nn_CausalSelfAttention (tensor-parallel over heads, 8 cores).

Contract: kernel(**inputs) takes FULL unsharded numpy inputs and returns the
FULL output [1, 2048, 1024] float32. Internally: shards over 8 NeuronCores
(2 heads each, Wq/Wk/Wv column-sharded, Wo row-sharded), runs one SPMD Bass
program via run_bass_kernel_spmd, and sums the 8 partial Wo products on the
host (the row-parallel unshard).

Final design (~118us HW exec vs the 143us phase-serial baseline; v2 was
PE-starved at 58% busy and HAM-throttled to 1.2GHz half the time):
  - quarter-streamed pipeline: x loads as 4x contiguous 1MB T-quarters on
    the sync HWDGE ring while first-quarter weights ride the scalar ring;
    each quarter's q/k/v projection, cosine-norm+rotary, and v transposes
    are woven as emission-order fillers into the previous chunk's
    attention so PE/ACT/DVE/GpSimd FIFOs match data-arrival order
  - warmup dummy matmuls cover preamble-to-first-data so the PE HAM
    clock gate opens before real work lands; end-game dummies bridge the
    final 1/Z chain
  - rotary pairs host-permuted to adjacent partitions: the partner swap
    is one DVE stream_shuffle (no PE hswap matmuls); rsqrt(sumsq) rides
    the same ACT Ln/Exp table set as the attention exp (no table switch)
  - S^T attention with 2-head PE-quadrant-packed S matmuls; AV packs
    [v|ones] so y and the softmax denominator Z come from one matmul;
    diagonal 512-blocks restrict S/exp/AV to the causal column range
    (exp via a strided 2-head AP) with the mask shrunk to one [128,128]
    affine_select triangle per head; 1/Z = exp(-ln Z) on ACT
  - PSUM (8 banks exactly): "ps" 2x[128,1024] S ring (shared by sumsq
    tiles and warmup), "h0" 2x[128,512] projection/transpose/Wo ring,
    "h1" 2x[128,512] y/Z accumulators -- Wo partials must NOT share the
    pyh ring (cross-tail DVE FIFO deadlock); outputs stored bf16 on
    both DMA rings and upcast host-side
Measured dead ends (kept out): fp8 anywhere (random-walk error > 2e-2
gate), DMA xbar transposes (issue on the ACT queue, blocking exps),
custom-DVE reciprocal (walrus rejects the ISA), bf16 matmul PSUM output
(bass asserts fp32), deeper/shallower S prefetch, LDWEIGHTS or matmul
HAM fillers, cross-chunk exp pre-hoisting (halves effective S-ring
depth). The residual gap to roofline is the 2-deep S ring coupling the
ACT exp stream (~58us) to PE jitter, plus fixed preamble/drain.
"""

import os
import sys
import types

import numpy as np
import ml_dtypes

for _p in ("/opt/trn_rl_repo", "/root/.axon_site/_ro/trn_rl_repo"):
    if os.path.isdir(_p) and _p not in sys.path:
        sys.path.append(_p)

import concourse.bass as bass
import concourse.mybir as mybir
import concourse.tile as tile
from concourse.bass_utils import run_bass_kernel_spmd

F32 = mybir.dt.float32
BF16 = mybir.dt.bfloat16
NPBF16 = ml_dtypes.bfloat16
NCORES = 8
T = 2048
D = 1024
NH = 16
HD = 64
HPC = NH // NCORES   # heads per core
EPC = HPC * HD       # projection cols per core
ATTN_SCALE = 0.12
NT = T // 512
NK = D // 128
NDUMMY = 84         # warmup matmuls (N=128) to open the HAM clock gate and
                     # keep the PE busy until the first x quarter lands

# stream_shuffle swaps adjacent partitions within each 32-partition quadrant;
# the host layout puts each rotary pair (d, d+32) on adjacent partitions.
SWAP_MASK = [(i ^ 1) for i in range(32)]

LAST = {}


def _register_ntff_hook():
    """Best-effort: register the axon NTFF profile hook if the image's antenv
    lacks axon_hooks (profiling only; compile/run work without it)."""
    try:
        import antenv.axon_hooks  # noqa: F401
        return
    except ImportError:
        pass
    try:
        import trn_agent_boot.trn_boot as tb

        mod = types.ModuleType("antenv.axon_hooks")
        holder = {}
        mod.set_axon_ntff_profile_hook = lambda h: holder.__setitem__("h", h)
        mod.get_axon_ntff_profile_hook = lambda: holder.get("h")
        sys.modules["antenv.axon_hooks"] = mod
        mod.set_axon_ntff_profile_hook(
            tb._ntff_profile_via_ctypes("/opt/axon/libaxon_pjrt.so")
        )
    except Exception:
        pass


def _split_ctrl_waits(nc, k_default=1):
    """The container's walrus build rejects instructions carrying more than one
    semaphore sync-wait; hoist extra waits onto single-wait NoOps that precede
    the instruction on the same engine queue (AND semantics preserved)."""
    n_nops = 0
    for f in nc.m.functions:
        for blk in f.blocks:
            new, changed = [], False
            for inst in list(blk.instructions):
                si = inst.sync_info
                waits = list(si.on_wait) if si is not None else []
                kmax = 1 if isinstance(inst, mybir.InstDrain) else k_default
                if len(waits) > kmax:
                    for k, w in enumerate(waits[:-kmax]):
                        nop = mybir.InstNoOp(name=f"{inst.name}-sw{k}", ins=[], outs=[])
                        nop.engine = inst.engine
                        nop.sync_info = mybir.SyncInfo(on_wait=[w], on_update=[])
                        new.append(nop)
                        n_nops += 1
                    inst.sync_info = mybir.SyncInfo(
                        on_wait=list(waits[-kmax:]), on_update=list(si.on_update)
                    )
                    changed = True
                new.append(inst)
            if changed:
                blk.instructions = new
    return n_nops


def _build_nc():
    nc = bass.Bass("TRN2", target_bir_lowering=False, debug=False, num_devices=NCORES)

    # x packed host-side as [128, (pair, i, t)]: pair-major so each 1MB pair
    # load is one fully contiguous 8KB-per-partition DMA.
    xP_d = nc.dram_tensor("xP", [128, 4 * 2 * T], BF16, kind="ExternalInput")
    wq_d = nc.dram_tensor("wq", [128, D], BF16, kind="ExternalInput")
    wk_d = nc.dram_tensor("wk", [128, D], BF16, kind="ExternalInput")
    wv_d = nc.dram_tensor("wv", [128, D], BF16, kind="ExternalInput")
    wo_d = nc.dram_tensor("wo", [EPC, D], BF16, kind="ExternalInput")
    rota_d = nc.dram_tensor("rota", [EPC, T], BF16, kind="ExternalInput")
    rotb_d = nc.dram_tensor("rotb", [EPC, T], BF16, kind="ExternalInput")
    hselw_d = nc.dram_tensor("hselw", [128, 128], BF16, kind="ExternalInput")
    out_d = nc.dram_tensor("out", [T, D], BF16, kind="ExternalOutput")

    with tile.TileContext(nc) as tc:
        with (
            tc.tile_pool(name="wt", bufs=1) as wt,
            tc.tile_pool(name="big", bufs=4) as big,      # x chunk-pairs
            tc.tile_pool(name="praw", bufs=3) as prawp,   # q/k/v raw
            tc.tile_pool(name="lwp", bufs=2) as lwp,      # ln(sumsq) staging
            tc.tile_pool(name="rwp", bufs=2) as rwp,      # rsqrt scales
            tc.tile_pool(name="rotp", bufs=2) as rotp,    # qrot/krot
            tc.tile_pool(name="sm", bufs=2) as smp,       # misc small tiles
            tc.tile_pool(name="at", bufs=1) as atp,       # pt2 / yt / ost
            # PSUM: "ps" 2x[128,1024] (4 banks) + "h0","h1" 2x[128,512] each
            # (2+2 banks) = 8 banks exactly.
            tc.tile_pool(name="ps", bufs=2, space="PSUM") as psp,
            tc.tile_pool(name="h0", bufs=2, space="PSUM") as h0p,
            tc.tile_pool(name="h1", bufs=2, space="PSUM") as h1p,
        ):
            # ---- ACT table warmup + PE warmup ----
            eps = wt.tile([128, 1], F32, tag="eps")
            nc.gpsimd.memset(eps[:], 1e-12)
            warm_in = wt.tile([128, 128], BF16, tag="warm_in")
            nc.gpsimd.memset(warm_in[:], 0.001)
            warm_f = wt.tile([128, 2], F32, tag="warm_f")
            warm_g = wt.tile([128, 2], F32, tag="warm_g")
            nc.gpsimd.memset(warm_f[:], 1.0)
            nc.scalar.activation(
                warm_g[:], warm_f[:], mybir.ActivationFunctionType.Ln,
                bias=eps[:],
            )
            nc.scalar.activation(
                warm_f[:], warm_g[:], mybir.ActivationFunctionType.Exp
            )
            # dummy matmuls: no data deps; keep the PE busy (and the HAM
            # un-throttled) from the preamble until the first x pair lands
            warm_ps = psp.tile([128, 1024], F32, tag="ps", name="warm_ps")
            for d in range(NDUMMY):
                nc.tensor.matmul(
                    warm_ps[:, 0:128], warm_in[:], warm_in[:],
                    start=True, stop=True,
                )

            # ---- constants / weights on the second HWDGE ring (scalar) ----
            wq_s = wt.tile([128, D], BF16, tag="wq")
            wk_s = wt.tile([128, D], BF16, tag="wk")
            wv_s = wt.tile([128, D], BF16, tag="wv")
            wo_s = wt.tile([EPC, D], BF16, tag="wo")
            rota = wt.tile([EPC, T], BF16, tag="rota")
            rotb = wt.tile([EPC, T], BF16, tag="rotb")
            hselw = wt.tile([128, 128], BF16, tag="hselw")
            # vext per 128-k block j: [v0^T | ones | v1^T] (192 cols); the AV
            # lhsT for head h is the contiguous 128-col window starting at
            # 64h, giving out partitions [y|Z] (h0) and [Z|y] (h1)
            vext = wt.tile([128, (T // 128) * 192], BF16, tag="vext")
            # only what the first quarter needs loads alongside xq0 (~0.8MB
            # of concurrent traffic); the rot-table tails and Wo ride the
            # sync ring behind the x quarters, off the critical window
            nc.scalar.dma_start(wq_s[:], wq_d[:])
            nc.scalar.dma_start(wk_s[:], wk_d[:])
            nc.scalar.dma_start(hselw[:], hselw_d[:])
            nc.scalar.dma_start(rota[:, 0:512], rota_d[:, 0:512])
            nc.scalar.dma_start(rotb[:, 0:512], rotb_d[:, 0:512])
            nc.scalar.dma_start(wv_s[:], wv_d[:])
            nc.gpsimd.memset(vext[:], 1.0)

            # ---- x streamed in four T-quarters: each 1MB load carries all
            # 8 contraction chunks for 512 time steps, so projections, norm,
            # and the chunk-c attention stream all start one quarter in ----
            xcp = []
            for a in range(4):
                t_ = big.tile([128, 2 * T], BF16, tag="big", name=f"xq_{a}")
                nc.sync.dma_start(t_[:], xP_d[:, 2 * T * a : 2 * T * (a + 1)])
                xcp.append(t_)
            nc.sync.dma_start(rota[:, 512:T], rota_d[:, 512:T])
            nc.sync.dma_start(rotb[:, 512:T], rotb_d[:, 512:T])
            nc.sync.dma_start(wo_s[:], wo_d[:])

            q_raw = prawp.tile([EPC, T], BF16, tag="praw", name="raw_q")
            k_raw = prawp.tile([EPC, T], BF16, tag="praw", name="raw_k")
            vT_raw = prawp.tile([EPC, T], BF16, tag="praw", name="raw_v")
            raws = {"q": q_raw, "k": k_raw, "v": vT_raw}
            wts = {"q": wq_s, "k": wk_s, "v": wv_s}

            acc_t = {}

            def proj_half(c, tname, half):
                # half a tensor-quarter (4 contraction chunks) per filler so
                # the PE bursts between S/AV groups stay short; the PSUM
                # copy drains while the next piece projects
                sl = slice(512 * c, 512 * (c + 1))
                if half == 0:
                    acc_t[(c, tname)] = h0p.tile(
                        [128, 512], F32, tag="h0", name=f"acc_{tname}_{c}"
                    )
                acc = acc_t[(c, tname)]
                for i in range(4 * half, 4 * half + 4):
                    nc.tensor.matmul(
                        acc[:],
                        wts[tname][:, 128 * i : 128 * (i + 1)],
                        xcp[c][:, 512 * i : 512 * (i + 1)],
                        start=(i == 0),
                        stop=(i == NK - 1),
                    )
                if half == 1:
                    if tname == "k":
                        nc.scalar.copy(raws[tname][:, sl], acc[:])
                    else:
                        nc.vector.tensor_copy(raws[tname][:, sl], acc[:])

            def proj_piece(c, tname):
                proj_half(c, tname, 0)
                proj_half(c, tname, 1)

            ident = wt.tile([128, 128], BF16, tag="ident")
            nc.gpsimd.memset(ident[:], 0.0)
            nc.gpsimd.affine_select(
                out=ident[:],
                in_=ident[:],
                compare_op=mybir.AluOpType.not_equal,
                fill=1.0,
                base=0,
                pattern=[[-1, 128]],
                channel_multiplier=1,
            )

            # ---- cosine-norm + scale + rotary, one 512-chunk at a time ----
            # rsqrt(sumsq) = exp(-0.5 ln(sumsq)); Ln and Exp share the
            # attention Exp table set (no table switch). Rotary partner swap
            # is a DVE stream_shuffle (host permuted pairs to adjacency).
            # Chunk-interleaved q/k so chunk 0 of both is ready earliest and
            # the chunk-0 S/exp stream starts while later chunks normalize.
            rw_t = {}
            rot_t = {}
            for tname in ("q", "k"):
                rw_t[tname] = rwp.tile([128, T], BF16, tag="rw",
                                       name=f"rw_{tname}")
                rot_t[tname] = rotp.tile([EPC, T], BF16, tag="rot",
                                         name=f"rot_{tname}")

            def norm_chunk(raw, tname, n):
                # chunk 0 is the critical prologue path: its elementwise ops
                # ride the faster DVE, and it is processed in two 256-col
                # slivers so the copy/shuffle/sumsq/Ln/Exp/rotary stages
                # pipeline (~1us earlier first exp); later chunks run full
                # width with the squares/rotb muls offloaded to GpSimd
                fast = n == 0
                mul_eng = nc.vector if fast else nc.gpsimd
                w = 256 if fast else 512
                for s in range(512 // w):
                    sl = slice(512 * n + w * s, 512 * n + w * (s + 1))
                    sw = smp.tile([128, w], BF16, name=f"sw_{tname}_{n}_{s}",
                                  tag="sw", bufs=4)
                    nc.vector.stream_shuffle(sw[:], raw[:, sl], SWAP_MASK)
                    sq = smp.tile([128, w], BF16, name=f"sq_{tname}_{n}_{s}",
                                  tag="sqm", bufs=4)
                    mul_eng.tensor_mul(sq[:], raw[:, sl], raw[:, sl])
                    ssb = psp.tile([128, w], F32, name=f"ssb_{tname}_{n}_{s}",
                                   tag="ps")
                    nc.tensor.matmul(ssb[:], hselw[:], sq[:], start=True,
                                     stop=True)
                    lw = smp.tile([128, w], F32, name=f"lw_{tname}_{n}_{s}",
                                  tag="lw", bufs=4)
                    nc.scalar.activation(
                        lw[:], ssb[:], mybir.ActivationFunctionType.Ln,
                        bias=eps[:],
                    )
                    rw = rw_t[tname]
                    nc.scalar.activation(
                        rw[:, sl], lw[:], mybir.ActivationFunctionType.Exp,
                        scale=-0.5,
                    )
                    t2 = smp.tile([128, w], BF16, name=f"t2_{tname}_{n}_{s}",
                                  tag="t2", bufs=4)
                    mul_eng.tensor_mul(t2[:], sw[:], rotb[:, sl])
                    t1 = smp.tile([128, w], BF16, name=f"t1_{tname}_{n}_{s}",
                                  tag="t1", bufs=4)
                    nc.vector.tensor_mul(t1[:], raw[:, sl], rota[:, sl])
                    nc.vector.tensor_add(t1[:], t1[:], t2[:])
                    nc.vector.tensor_mul(rot_t[tname][:, sl], t1[:],
                                         rw[:, sl])

            qrot = rot_t["q"]
            krot = rot_t["k"]

            # ---- attention pieces (S^T layout) ----
            pt_tiles = {}

            def emit_s(c, j):
                # ps2[:, (h, q)]: partition = k-time within block j. Diagonal
                # blocks only compute the causal column range [128m, 512).
                m = j - 4 * c
                lo = 128 * m if m > 0 else 0
                cq = slice(512 * c + lo, 512 * (c + 1))
                ps2 = psp.tile([128, 1024], F32, tag="ps", name=f"s_{c}_{j}")
                for h in range(HPC):
                    hs = slice(64 * h, 64 * (h + 1))
                    nc.tensor.matmul(
                        ps2[:, 512 * h + lo : 512 * (h + 1)],
                        krot[hs, 128 * j : 128 * (j + 1)],
                        qrot[hs, cq],
                        start=True,
                        stop=True,
                        tile_position=(64 * h, 0),
                    )
                return ps2

            def emit_exp(c, j, ps2):
                m = j - 4 * c
                # 16-deep: with the attention phase PE-bound (~1.35us/block
                # of S+AV vs ~1.15us of exp), AVs lag the exp stream through
                # a chunk; at 8 slots the exp for block j stalls on the AV
                # of block j-8 freeing its attention-weight tile
                pt2 = atp.tile([128, 1024], BF16, tag="pt", bufs=16,
                               name=f"p_{c}_{j}")
                if m > 0:
                    src = ps2.rearrange("p (h q) -> p h q", h=2)[:, :, 128 * m : 512]
                    dst = pt2.rearrange("p (h q) -> p h q", h=2)[:, :, 128 * m : 512]
                else:
                    src, dst = ps2[:], pt2[:]
                nc.scalar.activation(
                    dst, src, mybir.ActivationFunctionType.Exp,
                    scale=ATTN_SCALE,
                )
                if m >= 0:
                    # causal mask only on the [128,128] diagonal triangle
                    for h in range(HPC):
                        dsl = slice(512 * h + 128 * m, 512 * h + 128 * m + 128)
                        nc.gpsimd.affine_select(
                            out=pt2[:, dsl],
                            in_=pt2[:, dsl],
                            compare_op=mybir.AluOpType.is_ge,
                            fill=0.0,
                            base=0,
                            pattern=[[1, 128]],
                            channel_multiplier=-1,
                        )
                pt_tiles[(c, j)] = pt2

            def emit_av(c, j, pyh, nts):
                m = j - 4 * c
                lo = 128 * m if m > 0 else 0
                pt2 = pt_tiles.pop((c, j))
                for h in range(HPC):
                    nc.tensor.matmul(
                        pyh[h][:, lo:512],
                        vext[:, 192 * j + 64 * h : 192 * j + 64 * h + 128],
                        pt2[:, 512 * h + lo : 512 * (h + 1)],
                        start=(j == 0),
                        stop=(j == nts - 1),
                        skip_group_check=True,
                    )

            # v -> natural layout: PE transpose into a PSUM half-bank, then
            # two DVE copies into the [v0|ones|v1] vext window
            def emit_vtp(j):
                # h0 only: pyh lives on h1, and a vtp slot there would make
                # the first AV of a chunk wait for the last transpose's reads
                tp_ = h0p.tile([128, 128], BF16, tag="h0", name=f"vtp_{j}")
                nc.tensor.transpose(
                    tp_[:], vT_raw[:, 128 * j : 128 * (j + 1)], ident[:]
                )
                nc.vector.tensor_copy(
                    vext[:, 192 * j : 192 * j + 64], tp_[:, 0:64]
                )
                nc.vector.tensor_copy(
                    vext[:, 192 * j + 128 : 192 * j + 192], tp_[:, 64:128]
                )

            def emit_quarter(c):
                # q,k project first, their norms start while v projects
                proj_piece(c, "q")
                proj_piece(c, "k")
                norm_chunk(q_raw, "q", c)
                norm_chunk(k_raw, "k", c)
                proj_piece(c, "v")
                for j in range(4 * c, 4 * c + 4):
                    emit_vtp(j)

            # ---- attention core / tails ----
            def attention_core(c, skip_lead=0, fillers=()):
                # Software-pipelined: S/exp for block j+PF emitted before
                # mask/AV of block j so PE/ACT/GpSimd all stream. `fillers`
                # are emission callbacks (previous tail's Wo matmuls, the
                # next quarter's projection pieces) woven one per block so
                # the PE always has ready work between S/AV groups and the
                # ACT exp stream never starves behind a long projection.
                PF = 4
                nts = 4 * c + 4
                fillers = list(fillers)
                pyh = [
                    h1p.tile([128, 512], F32, name=f"py0_{c}", tag="h1"),
                    h1p.tile([128, 512], F32, name=f"py1_{c}", tag="h1"),
                ]
                for j in range(skip_lead, min(PF, nts)):
                    emit_exp(c, j, emit_s(c, j))
                if fillers:
                    fillers.pop(0)()
                for j in range(nts):
                    if skip_lead <= j + PF < nts:
                        emit_exp(c, j + PF, emit_s(c, j + PF))
                    emit_av(c, j, pyh, nts)
                    if fillers:
                        fillers.pop(0)()
                while fillers:
                    fillers.pop(0)()
                return pyh

            def tail_head(c, pyh, act_recip):
                # Emitted BEFORE the next chunk's core so the 1/Z chain runs
                # during the chunk handover instead of queueing behind the
                # next exps. 1/Z rides ACT (exp(-ln Z), same table set) when
                # ACT has slack (first/last chunk), the DVE reciprocal when
                # ACT is the binding stream mid-kernel.
                # y/Z locations: h0 -> y parts 0-63 (pyh0), Z parts 64-127;
                #                h1 -> Z parts 0-63 (pyh1), y parts 64-127.
                zcat = smp.tile([128, 512], F32, name=f"zcat_{c}", tag="zcat",
                                bufs=2)
                zal = smp.tile([128, 512], F32, name=f"zal_{c}", tag="zal",
                               bufs=2)
                nc.vector.tensor_copy(zcat[0:64, :], pyh[0][64:128, :])
                nc.vector.tensor_copy(zcat[64:128, :], pyh[1][0:64, :])
                if act_recip:
                    zlog = smp.tile([128, 512], F32, name=f"zlog_{c}",
                                    tag="zlog", bufs=2)
                    nc.scalar.activation(
                        zlog[:], zcat[:], mybir.ActivationFunctionType.Ln,
                        bias=eps[:],
                    )
                    nc.scalar.activation(
                        zal[:], zlog[:], mybir.ActivationFunctionType.Exp,
                        scale=-1.0,
                    )
                else:
                    nc.vector.reciprocal(zal[:], zcat[:])
                yt = atp.tile([128, 512], BF16, name=f"yt_{c}", tag="yt",
                              bufs=2)
                nc.vector.tensor_mul(yt[0:64, :], pyh[0][0:64, :],
                                     zal[0:64, :])
                nc.vector.tensor_mul(yt[64:128, :], pyh[1][64:128, :],
                                     zal[64:128, :])
                return yt

            def tail_rest(c, yt, last=False):
                # Wo partials live in the h0 half-bank ring only (sharing
                # with pyh's h1 ring would deadlock the DVE FIFO across
                # consecutive tails).
                for mi in range(4):
                    ms = slice(128 * mi, 128 * (mi + 1))
                    ost = atp.tile([128, D], BF16, name=f"ost_{c}_{mi}",
                                   tag="ost", bufs=3)
                    r0 = 512 * c + 128 * mi
                    for nn in range(2):
                        po = h0p.tile([128, 512], F32, tag="h0",
                                      name=f"po_{c}_{mi}_{nn}")
                        nc.tensor.matmul(
                            po[:],
                            yt[:, ms],
                            wo_s[:, 512 * nn : 512 * (nn + 1)],
                            start=True,
                            stop=True,
                        )
                        osl = slice(512 * nn, 512 * (nn + 1))
                        if last and (mi + nn) % 2 == 1:
                            nc.scalar.copy(ost[:, osl], po[:])
                        else:
                            nc.vector.tensor_copy(ost[:, osl], po[:])
                    eng = nc.sync if mi % 2 == 0 else nc.scalar
                    eng.dma_start(out_d[r0 : r0 + 128, :], ost[:])

            # ---- quarter-streamed emission: quarter c's projection/norm/
            # transposes are woven into the previous chunks' attention so
            # every engine's FIFO order matches data-arrival order and the
            # PE never runs a long projection burst that starves ACT ----
            # quarter 0 unrolled: the chunk-0 S/exp go ahead of the v
            # transposes in the PE FIFO (AV(0) doesn't run until core(0))
            proj_piece(0, "q")
            proj_piece(0, "k")
            norm_chunk(q_raw, "q", 0)
            norm_chunk(k_raw, "k", 0)
            proj_piece(0, "v")
            for j in range(4):
                emit_exp(0, j, emit_s(0, j))
            for j in range(4):
                emit_vtp(j)
            emit_quarter(1)
            prev = attention_core(0, skip_lead=4)
            yt0 = tail_head(0, prev, act_recip=True)
            cur1 = attention_core(1, fillers=[
                lambda: tail_rest(0, yt0),
                lambda: proj_half(2, "q", 0),
                lambda: proj_half(2, "q", 1),
                lambda: proj_half(2, "k", 0),
                lambda: proj_half(2, "k", 1),
                lambda: norm_chunk(q_raw, "q", 2),
                lambda: norm_chunk(k_raw, "k", 2),
                lambda: proj_half(2, "v", 0),
                lambda: proj_half(2, "v", 1),
                lambda: [emit_vtp(j) for j in range(8, 12)],
            ])
            yt1 = tail_head(1, cur1, act_recip=True)
            cur2 = attention_core(2, fillers=[
                lambda: tail_rest(1, yt1),
                lambda: proj_half(3, "q", 0),
                lambda: proj_half(3, "q", 1),
                lambda: proj_half(3, "k", 0),
                lambda: proj_half(3, "k", 1),
                lambda: norm_chunk(q_raw, "q", 3),
                lambda: norm_chunk(k_raw, "k", 3),
                lambda: proj_half(3, "v", 0),
                lambda: proj_half(3, "v", 1),
                lambda: [emit_vtp(j) for j in range(12, 16)],
            ])
            yt2 = tail_head(2, cur2, act_recip=True)
            cur3 = attention_core(3, fillers=[
                lambda: tail_rest(2, yt2),
            ])
            yt3 = tail_head(3, cur3, act_recip=True)
            # end-game dummies: the PE otherwise idles ~5us while the final
            # 1/Z chain runs, re-throttling the clock for the last Wo blocks
            end_ps = psp.tile([128, 1024], F32, tag="ps", name="end_ps")
            for d in range(48):
                nc.tensor.matmul(
                    end_ps[:, 0:128], warm_in[:], warm_in[:],
                    start=True, stop=True,
                )
            tail_rest(3, yt3, last=True)

    return nc


_NC = None
_NC_SPLIT = False


def _host_shards(x, Wq, Wk, Wv, Wo, s_qk):
    x = np.asarray(x, dtype=np.float32)
    Wq = np.asarray(Wq, dtype=np.float32)
    Wk = np.asarray(Wk, dtype=np.float32)
    Wv = np.asarray(Wv, dtype=np.float32)
    Wo = np.asarray(Wo, dtype=np.float32)
    s_qk = np.asarray(s_qk, dtype=np.float32)

    xT = np.ascontiguousarray(x.reshape(T, D).T).astype(NPBF16)
    # pack as [128, (quarter, i, t)]: each T-quarter is one contiguous 1MB
    # DMA carrying all 8 contraction chunks for 512 time steps
    xP = np.ascontiguousarray(
        xT.reshape(8, 128, 4, 512).transpose(1, 2, 0, 3).reshape(128, 8 * T)
    )

    dim_q = HD // 4
    freq = (1.0 / 1024.0) ** np.linspace(0.0, 1.0, dim_q, dtype=np.float32)
    freq = np.concatenate([freq, np.zeros(dim_q, np.float32)])
    theta = np.arange(T, dtype=np.float32)[:, None] * freq[None, :]
    cosT = np.cos(theta).T.astype(np.float32)
    sinT = np.sin(theta).T.astype(np.float32)
    A64 = np.concatenate([cosT, cosT], 0)          # [64, T]
    B64 = np.concatenate([sinT, -sinT], 0)         # [64, T]
    s_eff = s_qk * np.float32(np.sqrt(D))

    # per-head partition permutation: device partition j holds source dim
    # d(j) = (j%2)*32 + j//2, so rotary pairs (d, d+32) sit on (2r, 2r+1)
    dperm = np.array([(j % 2) * 32 + j // 2 for j in range(HD)], np.int64)
    dperm_sw = dperm[np.arange(HD) ^ 1]

    hselw = np.zeros((128, 128), np.float32)
    for h in range(HPC):
        hselw[64 * h : 64 * (h + 1), 64 * h : 64 * (h + 1)] = 1.0
    hselw = hselw.astype(NPBF16)

    def wlayout(w):
        # device lhsT chunk i = w_dev[:, 128*i:128*(i+1)] must equal
        # W[128*i + p, f]; store as [p, (i f)] so the DRAM load is contiguous
        return np.ascontiguousarray(
            w.reshape(NK, 128, EPC).transpose(1, 0, 2).reshape(128, NK * EPC)
        ).astype(NPBF16)

    in_maps = []
    for c in range(NCORES):
        cols = slice(EPC * c, EPC * (c + 1))
        wq_c = Wq[:, cols].reshape(D, HPC, HD)[:, :, dperm].reshape(D, EPC)
        wk_c = Wk[:, cols].reshape(D, HPC, HD)[:, :, dperm].reshape(D, EPC)
        rota_rows, rotb_rows = [], []
        for h in range(HPC):
            s = s_eff[HPC * c + h]
            rota_rows.append(s[dperm][:, None] * A64[dperm])
            rotb_rows.append(s[dperm_sw][:, None] * B64[dperm])
        in_maps.append(
            {
                "xP": xP,
                "wq": wlayout(wq_c),
                "wk": wlayout(wk_c),
                "wv": wlayout(Wv[:, cols]),
                "wo": np.ascontiguousarray(Wo[EPC * c : EPC * (c + 1), :]).astype(NPBF16),
                "rota": np.concatenate(rota_rows, 0).astype(NPBF16),
                "rotb": np.concatenate(rotb_rows, 0).astype(NPBF16),
                "hselw": hselw,
            }
        )
    return in_maps


def _run_device(in_maps):
    global _NC, _NC_SPLIT
    _register_ntff_hook()
    if _NC is None:
        _NC = _build_nc()
    if not _NC_SPLIT:
        _split_ctrl_waits(_NC)
        _NC_SPLIT = True
    res = run_bass_kernel_spmd(_NC, in_maps, list(range(NCORES)))
    return (
        [np.asarray(r["out"]) for r in res.results],
        res.exec_time_ns,
        res.instructions_and_trace[1] if res.instructions_and_trace else None,
    )


def _worker(in_pkl, out_pkl):
    import pickle

    with open(in_pkl, "rb") as f:
        in_maps = pickle.load(f)
    outs, exec_ns, trace = _run_device(in_maps)
    with open(out_pkl, "wb") as f:
        pickle.dump({"outs": outs, "exec_time_ns": exec_ns, "trace": trace}, f)


def _run_subprocess(in_maps):
    import pickle
    import subprocess
    import tempfile

    d = tempfile.mkdtemp()
    in_pkl = os.path.join(d, "in.pkl")
    out_pkl = os.path.join(d, "out.pkl")
    with open(in_pkl, "wb") as f:
        pickle.dump(in_maps, f)
    here = os.path.dirname(os.path.abspath(__file__))
    code = (
        f"import sys; sys.path.insert(0, {here!r}); "
        f"import kernel; kernel._worker({in_pkl!r}, {out_pkl!r})"
    )
    subprocess.run([sys.executable, "-c", code], check=True, timeout=1800)
    with open(out_pkl, "rb") as f:
        out = pickle.load(f)
    return out["outs"], out["exec_time_ns"], out["trace"]


def _attempt(in_maps, use_subprocess):
    if use_subprocess:
        return _run_subprocess(in_maps)
    return _run_device(in_maps)


def kernel(x, Wq, Wk, Wv, Wo, s_qk):
    in_maps = _host_shards(x, Wq, Wk, Wv, Wo, s_qk)

    def total_of(outs):
        t = np.zeros((T, D), np.float64)
        for o in outs:
            t += o.astype(np.float64)
        return t

    # Run until two executions agree: device runs are deterministic, so a
    # mismatch flags the sporadic silent-corruption failure mode. Crashed
    # runs (NRT unrecoverable) poison this process's PJRT client, so later
    # attempts fall back to fresh subprocesses.
    results = []
    last_exc = None
    sub = False
    for attempt in range(5):
        try:
            outs, exec_ns, trace = _attempt(in_maps, sub)
        except Exception as e:
            last_exc = e
            sub = True
            continue
        t = total_of(outs)
        LAST["exec_time_ns"] = exec_ns
        LAST["trace"] = trace
        for tprev in results:
            denom = max(float(np.abs(tprev).max()), 1e-6)
            if float(np.abs(t - tprev).max()) <= 1e-4 * denom:
                return t.astype(np.float32).reshape(1, T, D)
        results.append(t)
    if results:
        return results[-1].astype(np.float32).reshape(1, T, D)
    raise last_exc
